# revision 16
# baseline (speedup 1.0000x reference)
# Trainium2 Bass kernel for BaseGumbelGraphNetwork message passing.
#
# Reference computation (B=4, N=512, D=2, H=64, O=2):
#   e1 = relu(cat(x_i, x_j) @ W_n2e.T + b_n2e)        [B,N,N,H]
#   e2 = relu(e1 @ W_e2e.T + b_e2e)                   [B,N,N,H]
#   s  = sum_j adj[i,j] * e2                          [B,N,H]
#   h  = relu(relu(s@W_e2n.T+b)@W_n2n.T+b)
#   out= relu(cat(x,h)@W_o1.T+b) @ W_o2.T + b         [B,N,O]
#
# Key structure: layer 1 factorizes over the (i,j) grid:
#   e1[b,i,j,:] = relu(A[b,i,:] + C[b,j,:] + b1),  A = x@Wi.T, C = x@Wj.T
# so the [B,N,N,2D] concat tensor is never materialized.
#
# Device layout (per core, i-dim sharded 8 ways -> 64 rows/core):
#   * units are (i-pair, batch): partitions = h stacked for two i's (2x64),
#     free dim = j (512). Units are processed in batch-pairs: two units share
#     one [128,1024] tile so the ACT / DVE instruction overheads amortize.
#   * per unit-pair:
#       2x DVE tensor_scalar  e1 = relu(C.T + (A_i + b1))   fp32->bf16
#       2x PE matmul          e2pre = blockdiag(W_e2e.T).T @ e1   (bf16, N=512)
#       1x ACT                e2 = relu(e2pre + b2)   PSUM -> SBUF bf16 [128,1024]
#       1x DVE tensor_tensor  scr = e2 * adj_bcast              [128,1024]
#       1x PE matmul (fused reduce): h1pre[:, q] += sum_j W_e2n_bd.T @ scr[:, j]
#          via a PSUM output access pattern that repeats 8 columns 64 times --
#          PSUM's per-element has_written accumulate sums all 512 j-columns
#          in hardware (verified: 8-column spacing avoids the RMW hazard that
#          corrupts a pure step-0 pattern).
#   * adj rows are partition-broadcast into SBUF by DMA (otherwise idle).
#   * the tiny node MLP at the end also runs on-device with block-diag matmuls.

import numpy as np

B, N, D, H, O = 4, 512, 2, 64, 2
NCORES = 8
IB = N // NCORES  # i rows per core = 64
Q = IB // 2       # i pairs per core = 32

FUSED_RED = True  # j-reduction fused into a PE matmul (HW-only semantics;
                  # CoreSim does not model repeated-AP PSUM accumulation)

_STATE = {}


def _build_nc():
    import concourse.mybir as mybir
    from concourse import bacc
    from concourse.tile import TileContext

    F32 = mybir.dt.float32
    FP16 = mybir.dt.float16   # e1 / W2-matmul path (better weight precision)
    BFL = mybir.dt.bfloat16   # e2 / mask / reduce path (full-rate ACT writes)
    AL = mybir.AluOpType
    AF = mybir.ActivationFunctionType

    nc = bacc.Bacc("TRN2", target_bir_lowering=False, debug=False,
                   num_devices=NCORES)

    def din(name, shape, dt=F32):
        return nc.dram_tensor(name, list(shape), dt, kind="ExternalInput").ap()

    xT = din("xT", (B, 2, N))            # x[b].T (d-major, all j)
    xtie = din("xtie", (B, 2, Q))        # x of even local i, transposed
    xtio = din("xtio", (B, 2, Q))        # x of odd local i, transposed
    xpair = din("xpair", (B, 4, Q))      # rows (e*2+d) for final concat layer
    adjr = din("adjr", (IB, N), BFL)    # this core's adjacency row block
    wjt2 = din("wjt2", (2, 128))         # [Wj.T | Wj.T]
    wit = din("wit", (2, H))             # Wi.T
    b1s = din("b1s", (128, 1))           # [b_n2e; b_n2e]
    b2s = din("b2s", (128, 1))           # [b_e2e; b_e2e]
    w2bd = din("w2bd", (128, 128), FP16)  # blockdiag(W_e2e.T, W_e2e.T)
    we2nbd = din("we2nbd", (128, 128), BFL)  # blockdiag(W_e2n.T, W_e2n.T)
    we2nbdf = din("we2nbdf", (128, 128))      # fp32 copy (non-fused path)
    be2ns = din("be2ns", (128, 1))
    wn2nbd = din("wn2nbd", (128, 128))
    bn2ns = din("bn2ns", (128, 1))
    wo1hbd = din("wo1hbd", (128, 128))
    wo1xbd = din("wo1xbd", (4, 128))
    bo1s = din("bo1s", (128, 1))
    wo2bd = din("wo2bd", (128, 4))
    bo2s = din("bo2s", (4, 1))

    out_d = nc.dram_tensor("out", [B, IB, O], F32, kind="ExternalOutput").ap()
    # out[b, 2q+e, o] <- OUT_sb[b][2e+o, q]
    out_re = out_d.rearrange("b (q e) o -> b (e o) q", e=2)

    with TileContext(nc) as tc:
        with (tc.tile_pool(name="wpool", bufs=1) as wp,
              tc.tile_pool(name="ctsp", bufs=B) as ctsp,
              tc.tile_pool(name="abp", bufs=B) as abp,
              tc.tile_pool(name="sp", bufs=B) as sp,
              tc.tile_pool(name="adjp", bufs=6) as adjp,
              tc.tile_pool(name="e1p", bufs=6) as e1p,
              tc.tile_pool(name="e2p", bufs=6) as e2p,
              tc.tile_pool(name="scrp", bufs=8) as scrp,
              tc.tile_pool(name="finp", bufs=2) as finp,
              tc.tile_pool(name="psp", bufs=3, space="PSUM") as psp,
              tc.tile_pool(name="hps", bufs=1, space="PSUM") as hps):

            def wload(ap_dram, shape, dt=F32, tag=None):
                t = wp.tile(list(shape), dt, tag=tag)
                nc.sync.dma_start(out=t[:], in_=ap_dram[:])
                return t

            wjt2_s = wload(wjt2, (2, 128), tag="wjt2")
            wit_s = wload(wit, (2, H), tag="wit")
            b1s_s = wload(b1s, (128, 1), tag="b1s")
            b2s_s = wload(b2s, (128, 1), tag="b2s")
            w2bd_s = wload(w2bd, (128, 128), FP16, tag="w2bd")
            we2nbd_s = wload(we2nbd, (128, 128), BFL, tag="we2nbd")
            we2nbdf_s = wload(we2nbdf, (128, 128), tag="we2nbdf")
            be2ns_s = wload(be2ns, (128, 1), tag="be2ns")
            wn2nbd_s = wload(wn2nbd, (128, 128), tag="wn2nbd")
            bn2ns_s = wload(bn2ns, (128, 1), tag="bn2ns")
            wo1hbd_s = wload(wo1hbd, (128, 128), tag="wo1hbd")
            wo1xbd_s = wload(wo1xbd, (4, 128), tag="wo1xbd")
            bo1s_s = wload(bo1s, (128, 1), tag="bo1s")
            wo2bd_s = wload(wo2bd, (128, 4), tag="wo2bd")
            bo2s_s = wload(bo2s, (4, 1), tag="bo2s")

            # fused-reduce accumulator: h1pre for all (b, q), 8 sub-columns
            # each; [128, B*Q*8] fp32 = 2 PSUM banks
            h1ps = hps.tile([128, B * Q * 8], F32, tag="h1ps")
            h1v = h1ps[:].rearrange("p (b q e) -> p b q e", b=B, e=8)

            # ---- per-batch setup: CTS (stacked C.T) and ABIAS (A + b1) ----
            CTS, AB, S, XP = [], [], [], []
            for b in range(B):
                xT_s = wp.tile([2, N], F32, tag=f"xT{b}")
                nc.sync.dma_start(out=xT_s[:], in_=xT[b])
                xtie_s = wp.tile([2, Q], F32, tag=f"xtie{b}")
                nc.sync.dma_start(out=xtie_s[:], in_=xtie[b])
                xtio_s = wp.tile([2, Q], F32, tag=f"xtio{b}")
                nc.sync.dma_start(out=xtio_s[:], in_=xtio[b])
                xpair_s = wp.tile([4, Q], F32, tag=f"xpair{b}")
                nc.sync.dma_start(out=xpair_s[:], in_=xpair[b])

                ps = psp.tile([128, 1024], F32, tag="ps")
                nc.tensor.matmul(ps[:, 0:512], lhsT=wjt2_s[:], rhs=xT_s[:],
                                 start=True, stop=True)
                cts = ctsp.tile([128, N], FP16, tag="cts")
                nc.scalar.copy(cts[:], ps[:, 0:512])

                ps2 = psp.tile([128, 1024], F32, tag="ps")
                nc.tensor.matmul(ps2[0:64, 0:Q], lhsT=wit_s[:], rhs=xtie_s[:],
                                 start=True, stop=True)
                nc.tensor.matmul(ps2[64:128, 0:Q], lhsT=wit_s[:], rhs=xtio_s[:],
                                 start=True, stop=True)
                ab = abp.tile([128, Q], F32, tag="ab")
                nc.vector.tensor_scalar_add(out=ab[:], in0=ps2[0:128, 0:Q],
                                            scalar1=b1s_s[:])
                s_t = sp.tile([128, Q], F32, tag="S")
                CTS.append(cts)
                AB.append(ab)
                S.append(s_t)
                XP.append(xpair_s)

            # ---- main loop: (i-pair q, batch-pair bp) ----
            # Software-pipelined emission: each engine executes its stream in
            # emission order, so cross-engine consumers are emitted behind
            # their producers: the mask TT of iteration q is emitted during
            # q+1, and the fused-reduce matmul during q+2. This keeps DVE
            # from stalling on ACT and the PE from stalling on DVE.
            def emit_tt(jobs):
                for adjrep_, e2m_, scrm_ in jobs:
                    nc.vector.tensor_tensor(
                        out=scrm_[:].rearrange("p (u j) -> p u j", u=2),
                        in0=e2m_[:].rearrange("p (u j) -> p u j", u=2),
                        in1=adjrep_, op=AL.mult)

            def emit_red(jobs):
                for b, q_, scrm_, k in jobs:
                    if FUSED_RED:
                        sl = h1v[:, b, q_:q_ + 1, :]             # [128, 1, 8]
                        nc.tensor.matmul(sl.broadcast_to((128, 64, 8)),
                                         lhsT=we2nbd_s[:],
                                         rhs=scrm_[:, 512 * k:512 * (k + 1)],
                                         start=True, stop=True)
                    else:
                        nc.vector.tensor_reduce(
                            out=S[b][:, q_:q_ + 1],
                            in_=scrm_[:, 512 * k:512 * (k + 1)],
                            axis=mybir.AxisListType.X, op=AL.add)

            tt_prev = []    # TT jobs from iteration q-1
            red_prev1 = []  # reduce jobs from q-1 (TT not yet emitted)
            red_prev2 = []  # reduce jobs from q-2 (TT emitted at q-1)
            for q in range(Q):
                adjt = adjp.tile([128, N], BFL, tag="adj")
                nc.sync.dma_start(
                    out=adjt[0:64, :],
                    in_=adjr[2 * q:2 * q + 1, :].partition_broadcast(64))
                nc.sync.dma_start(
                    out=adjt[64:128, :],
                    in_=adjr[2 * q + 1:2 * q + 2, :].partition_broadcast(64))
                adjrep = adjt[:].unsqueeze(1).broadcast_to((128, 2, 512))
                cur_tt, cur_red = [], []
                for bp in range(2):
                    b0, b1 = 2 * bp, 2 * bp + 1
                    e1m = e1p.tile([128, 1024], FP16, tag="e1")
                    for k, b in enumerate((b0, b1)):
                        nc.vector.tensor_scalar(
                            out=e1m[:, 512 * k:512 * (k + 1)], in0=CTS[b][:],
                            scalar1=AB[b][:, q:q + 1], scalar2=0.0,
                            op0=AL.add, op1=AL.max)
                    psm = psp.tile([128, 1024], F32, tag="ps")
                    nc.tensor.matmul(psm[:, 0:512], lhsT=w2bd_s[:],
                                     rhs=e1m[:, 0:512], start=True, stop=True)
                    nc.tensor.matmul(psm[:, 512:1024], lhsT=w2bd_s[:],
                                     rhs=e1m[:, 512:1024], start=True,
                                     stop=True)
                    e2m = e2p.tile([128, 1024], BFL, tag="e2")
                    nc.scalar.activation(e2m[:], psm[:], AF.Relu,
                                         bias=b2s_s[:])
                    scrm = scrp.tile([128, 1024], BFL, tag="scr")
                    cur_tt.append((adjrep, e2m, scrm))
                    for k, b in enumerate((b0, b1)):
                        cur_red.append((b, q, scrm, k))
                emit_tt(tt_prev)
                emit_red(red_prev2)
                tt_prev = cur_tt
                red_prev2 = red_prev1
                red_prev1 = cur_red
            emit_tt(tt_prev)
            emit_red(red_prev2)
            emit_red(red_prev1)

            # ---- final node MLP per batch (tiny) ----
            for b in range(B):
                if FUSED_RED:
                    h1pre = finp.tile([128, Q], F32, tag="h1pre")
                    nc.vector.tensor_reduce(out=h1pre[:], in_=h1v[:, b],
                                            axis=mybir.AxisListType.X,
                                            op=AL.add)
                    h1 = finp.tile([128, Q], F32, tag="h1")
                    nc.scalar.activation(h1[:], h1pre[:], AF.Relu,
                                         bias=be2ns_s[:])
                else:
                    ps = psp.tile([128, 1024], F32, tag="ps")
                    nc.tensor.matmul(ps[:, 0:Q], lhsT=we2nbdf_s[:],
                                     rhs=S[b][:], start=True, stop=True)
                    h1 = finp.tile([128, Q], F32, tag="h1")
                    nc.scalar.activation(h1[:], ps[:, 0:Q], AF.Relu,
                                         bias=be2ns_s[:])

                ps2 = psp.tile([128, 1024], F32, tag="ps")
                nc.tensor.matmul(ps2[:, 0:Q], lhsT=wn2nbd_s[:], rhs=h1[:],
                                 start=True, stop=True)
                h2 = finp.tile([128, Q], F32, tag="h2")
                nc.scalar.activation(h2[:], ps2[:, 0:Q], AF.Relu,
                                     bias=bn2ns_s[:])

                ps3 = psp.tile([128, 1024], F32, tag="ps")
                nc.tensor.matmul(ps3[:, 0:Q], lhsT=wo1hbd_s[:], rhs=h2[:],
                                 start=True, stop=False)
                nc.tensor.matmul(ps3[:, 0:Q], lhsT=wo1xbd_s[:], rhs=XP[b][:],
                                 start=False, stop=True)
                h3 = finp.tile([128, Q], F32, tag="h3")
                nc.scalar.activation(h3[:], ps3[:, 0:Q], AF.Relu,
                                     bias=bo1s_s[:])

                ps4 = psp.tile([128, 1024], F32, tag="ps")
                nc.tensor.matmul(ps4[0:4, 0:Q], lhsT=wo2bd_s[:], rhs=h3[:],
                                 start=True, stop=True)
                outs = finp.tile([4, Q], F32, tag="outs")
                nc.scalar.activation(outs[:], ps4[0:4, 0:Q], AF.Identity,
                                     bias=bo2s_s[:])
                nc.sync.dma_start(out=out_re[b], in_=outs[:])

    nc.compile()
    return nc


def _get_nc():
    if "nc" not in _STATE:
        _STATE["nc"] = _build_nc()
    return _STATE["nc"]


def _prep_maps(inputs):
    import ml_dtypes
    bfl = ml_dtypes.bfloat16
    fp16 = np.float16
    f32 = np.float32

    x = np.ascontiguousarray(np.asarray(inputs["input"], f32))      # [B,N,D]
    adj = np.ascontiguousarray(np.asarray(inputs["adj"], f32))      # [N,N]
    W_n2e = np.asarray(inputs["W_n2e"], f32)   # [H, 2D]
    b_n2e = np.asarray(inputs["b_n2e"], f32)
    W_e2e = np.asarray(inputs["W_e2e"], f32)
    b_e2e = np.asarray(inputs["b_e2e"], f32)
    W_e2n = np.asarray(inputs["W_e2n"], f32)
    b_e2n = np.asarray(inputs["b_e2n"], f32)
    W_n2n = np.asarray(inputs["W_n2n"], f32)
    b_n2n = np.asarray(inputs["b_n2n"], f32)
    W_o1 = np.asarray(inputs["W_o1"], f32)     # [H, D+H]
    b_o1 = np.asarray(inputs["b_o1"], f32)
    W_o2 = np.asarray(inputs["W_o2"], f32)     # [O, H]
    b_o2 = np.asarray(inputs["b_o2"], f32)

    Wi, Wj = W_n2e[:, :D], W_n2e[:, D:]

    def bd(w):  # blockdiag(w, w)
        r, c = w.shape
        z = np.zeros((2 * r, 2 * c), f32)
        z[:r, :c] = w
        z[r:, c:] = w
        return z

    shared = {
        "xT": x.transpose(0, 2, 1),
        "wjt2": np.concatenate([Wj.T, Wj.T], axis=1),
        "wit": Wi.T,
        "b1s": np.concatenate([b_n2e, b_n2e]).reshape(128, 1),
        "b2s": np.concatenate([b_e2e, b_e2e]).reshape(128, 1),
        "w2bd": bd(W_e2e.T).astype(fp16),
        "we2nbd": bd(W_e2n.T).astype(bfl),
        "we2nbdf": bd(W_e2n.T),
        "be2ns": np.concatenate([b_e2n, b_e2n]).reshape(128, 1),
        "wn2nbd": bd(W_n2n.T),
        "bn2ns": np.concatenate([b_n2n, b_n2n]).reshape(128, 1),
        "wo1hbd": bd(W_o1[:, D:].T),
        "wo1xbd": bd(W_o1[:, :D].T),           # [4, 128]
        "bo1s": np.concatenate([b_o1, b_o1]).reshape(128, 1),
        "wo2bd": bd(W_o2.T),                   # [128, 4]
        "bo2s": np.concatenate([b_o2, b_o2]).reshape(4, 1),
    }
    shared = {k: np.ascontiguousarray(v, dtype=v.dtype)
              for k, v in shared.items()}

    maps = []
    for c in range(NCORES):
        sl = slice(c * IB, (c + 1) * IB)
        xc = x[:, sl]                                    # [B, IB, D]
        m = dict(shared)
        m["adjr"] = adj[sl].astype(bfl)
        m["xtie"] = np.ascontiguousarray(xc[:, 0::2].transpose(0, 2, 1))
        m["xtio"] = np.ascontiguousarray(xc[:, 1::2].transpose(0, 2, 1))
        m["xpair"] = np.ascontiguousarray(
            xc.reshape(B, Q, 2 * D).transpose(0, 2, 1))  # rows e*2+d
        maps.append(m)
    return maps


def run(inputs, trace=False, **kw):
    from concourse.bass_utils import run_bass_kernel_spmd
    nc = _get_nc()
    maps = _prep_maps(inputs)
    res = run_bass_kernel_spmd(nc, maps, list(range(NCORES)), trace=trace, **kw)
    out = np.concatenate([res.results[c]["out"] for c in range(NCORES)], axis=1)
    return np.ascontiguousarray(out, dtype=np.float32), res


def kernel(**inputs):
    out, _ = run(inputs, trace=False)
    return out
